# revision 81
# baseline (speedup 1.0000x reference)
"""BERT single-layer kernel for 8 Trainium2 NeuronCores.

Sharding: core c handles batch b=c//2, query-token half q=c%2 (256 of 512
tokens). Each core computes K/V for its batch's full 512 tokens (duplicated
within the pair, avoiding collectives). The dominant enc@Wproj [256,768]x
[768,32000] streams bf16 weights once; both 128-token blocks' logits live in
SBUF bf16, log-softmax is computed online via per-tile exp row-sums.
"""
import sys

sys.path.insert(0, "/opt/trn_rl_repo")

import numpy as np
import ml_dtypes

import concourse.bass as bass
import concourse.bacc as bacc
import concourse.mybir as mybir
import concourse.tile as tile
from concourse.bass import IndirectOffsetOnAxis
from concourse.bass_utils import run_bass_kernel_spmd

P = 128
B, T, V, D, DH, H = 4, 512, 32000, 768, 768, 4
DC = D // P          # 6 chunks of the feature dim
TQ = 256             # query tokens per core
NV = 63              # vocab tiles: 62*512 + 256
SCALE = float(T) ** -0.5
EPS = 1e-5

f32 = mybir.dt.float32
bf16 = mybir.dt.bfloat16
i32 = mybir.dt.int32

AF = mybir.ActivationFunctionType
AX = mybir.AxisListType
OP = mybir.AluOpType


def _bcast(nc, out_tile, dram, offset, width):
    """Broadcast `width` contiguous elements at `offset` of 1-D dram tensor
    across all 128 partitions of out_tile."""
    nc.gpsimd.dma_start(out=out_tile, in_=bass.AP(dram, offset, [[0, P], [1, width]]))


def build_program():
    nc = bacc.Bacc(None)

    # ---- dram I/O ----
    idx_d = nc.dram_tensor("idx", [T], i32, kind="ExternalInput")
    te_d = nc.dram_tensor("te", [V, D], f32, kind="ExternalInput")
    peseg_d = nc.dram_tensor("peseg", [T, D], f32, kind="ExternalInput")
    keep_d = nc.dram_tensor("keep", [T], f32, kind="ExternalInput")
    fill_d = nc.dram_tensor("fill", [T], f32, kind="ExternalInput")
    wq_d = nc.dram_tensor("wq", [H, D, DH], bf16, kind="ExternalInput")
    wk_d = nc.dram_tensor("wk", [H, D, DH], bf16, kind="ExternalInput")
    wv_d = nc.dram_tensor("wv", [H, D, DH], bf16, kind="ExternalInput")
    bq_d = nc.dram_tensor("bq", [H, DH], f32, kind="ExternalInput")
    bk_d = nc.dram_tensor("bk", [H, DH], f32, kind="ExternalInput")
    bv_d = nc.dram_tensor("bv", [H, DH], f32, kind="ExternalInput")
    wo_d = nc.dram_tensor("wo", [H * DH, D], bf16, kind="ExternalInput")
    bo_d = nc.dram_tensor("bo", [D], f32, kind="ExternalInput")
    w1_d = nc.dram_tensor("w1", [D, DH], bf16, kind="ExternalInput")
    b1_d = nc.dram_tensor("b1", [DH], f32, kind="ExternalInput")
    w2_d = nc.dram_tensor("w2", [DH, D], bf16, kind="ExternalInput")
    b2_d = nc.dram_tensor("b2", [D], f32, kind="ExternalInput")
    lng_d = {k: nc.dram_tensor(f"ln{k}g", [D], f32, kind="ExternalInput")
             for k in ("e", "a", "n")}
    lnb_d = {k: nc.dram_tensor(f"ln{k}b", [D], f32, kind="ExternalInput")
             for k in ("e", "a", "n")}
    wp_d = nc.dram_tensor("wp", [D, V], bf16, kind="ExternalInput")
    bp_d = nc.dram_tensor("bp", [V], bf16, kind="ExternalInput")
    wc_d = nc.dram_tensor("wc", [D, 2], bf16, kind="ExternalInput")
    bc_d = nc.dram_tensor("bc", [2], f32, kind="ExternalInput")
    eye_d = nc.dram_tensor("eye", [P, P], f32, kind="ExternalInput")

    logits_d = nc.dram_tensor("logits", [TQ, V], mybir.dt.float16,
                              kind="ExternalOutput")
    cls_d = nc.dram_tensor("cls", [P, 2], f32, kind="ExternalOutput")

    # SPMD: one program for all 8 cores. Query tokens always sit at positions
    # 0..255 — the host rolls token order by -256 for q=1 cores (attention is
    # permutation-invariant over the key/value axis since mask/pe roll along).

    wq_r = [wq_d[h].rearrange("(dc p) e -> p dc e", p=P) for h in range(H)]
    wk_r = [wk_d[h].rearrange("(dc p) e -> p dc e", p=P) for h in range(H)]
    wv_r = [wv_d[h].rearrange("(dc p) e -> p dc e", p=P) for h in range(H)]
    wo_r = wo_d.rearrange("(hc p) d -> p hc d", p=P)
    w1_r = w1_d.rearrange("(dc p) e -> p dc e", p=P)
    w2_r = w2_d.rearrange("(dc p) e -> p dc e", p=P)
    wp_r = wp_d.rearrange("(dc p) v -> p dc v", p=P)
    wc_r = wc_d.rearrange("(dc p) c -> p dc c", p=P)
    idx_r = idx_d.rearrange("(sb p) -> p sb", p=P)
    peseg_r = peseg_d.rearrange("(sb p) d -> p sb d", p=P)
    bqk_r = {"q": bq_d.rearrange("h (ec p) -> p h ec", p=P),
             "k": bk_d.rearrange("h (ec p) -> p h ec", p=P)}

    with tile.TileContext(nc) as tc:
        with tc.tile_pool(name="const", bufs=1) as const, \
             tc.tile_pool(name="misc", bufs=1) as misc, \
             tc.tile_pool(name="enc", bufs=1) as encp, \
             tc.tile_pool(name="psbig", bufs=4, space="PSUM") as psbig, \
             tc.tile_pool(name="psmid", bufs=2, space="PSUM") as psmid, \
             tc.tile_pool(name="pstr", bufs=2, space="PSUM") as pstr:

            idx_sb = const.tile([P, T // P], i32)
            nc.sync.dma_start(out=idx_sb, in_=idx_r)
            ident = const.tile([P, P], f32)
            nc.sync.dma_start(out=ident, in_=eye_d[:])
            eps_t = const.tile([P, 1], f32)
            nc.vector.memset(eps_t, EPS)
            # dummy op: pull the sqrt ACT table load off the first LN's path
            warm = const.tile([P, 1], f32)
            nc.scalar.activation(warm, eps_t, AF.Sqrt)
            keep_sb = const.tile([P, T], f32)
            fill_sb = const.tile([P, T], f32)
            bqk_sb = {}
            for k in ("q", "k"):
                bqk_sb[k] = const.tile([P, H, DC], f32, name=f"b{k}sb", tag=f"b{k}sb")
            wcls_sb = const.tile([P, DC, 2], bf16)
            bcls_sb = const.tile([P, 2], f32)

            encTh2 = [encp.tile([P, DC, P], bf16, tag="encT0", name="encT0"),
                      encp.tile([P, DC, P], bf16, tag="encT1", name="encT1")]
            sums = misc.tile([P, 2, 64], f32)


            def layernorm(tp, x, g_bc, b_bc, lnexp=False):
                """LN over free axis of x [P, D] f32, in place (then *g+b).

                lnexp=True computes rsqrt as exp(-0.5*ln(v+eps)) — keeps ACT in
                the exp/ln table set (no sqrt-set swap) and frees DVE of the
                reciprocal; used for the LNs adjacent to exp/gelu phases."""
                st = tp.tile([P, 3, 6], f32, tag="ln_st")
                for i in range(3):
                    nc.vector.bn_stats(out=st[:, i], in_=x[:, i * 256:(i + 1) * 256])
                mv = tp.tile([P, 2], f32, tag="ln_mv")
                nc.vector.bn_aggr(out=mv, in_=st)
                rstd = tp.tile([P, 1], f32, tag="ln_rstd")
                if lnexp:
                    nc.scalar.activation(rstd, mv[:, 1:2], AF.Ln, bias=eps_t)
                    nc.scalar.activation(rstd, rstd, AF.Exp, scale=-0.5)
                else:
                    nc.scalar.activation(rstd, mv[:, 1:2], AF.Sqrt, bias=eps_t)
                    nc.vector.reciprocal(rstd, rstd)
                nc.vector.tensor_scalar(x, x, scalar1=mv[:, 0:1], scalar2=rstd,
                                        op0=OP.subtract, op1=OP.mult)
                nc.vector.tensor_mul(x, x, g_bc)
                nc.vector.tensor_add(x, x, b_bc)

            def transpose_to(dst_slices, x):
                """x [P(tok), D] f32 -> dst[:, dc, tok-slice] bf16 transposed.

                PSUM->SBUF copies go to ACT (idle in the LN stages) so they
                run parallel to the DVE layernorm chains."""
                for dc in range(DC):
                    pt = pstr.tile([P, P], f32, tag="tr")
                    nc.tensor.transpose(pt, x[:, dc * P:(dc + 1) * P], ident)
                    nc.scalar.copy(dst_slices(dc), pt)

            ws_ctx = tc.tile_pool(name="wstream", bufs=3)
            wstream = ws_ctx.__enter__()
            with tc.tile_pool(name="chain", bufs=1) as chain, \
                 tc.tile_pool(name="bc", bufs=3) as bc, \
                 tc.tile_pool(name="tmp", bufs=2) as tmp:

                embTh = [chain.tile([P, DC, TQ], bf16, tag="embT0", name="embT0"),
                         chain.tile([P, DC, TQ], bf16, tag="embT1", name="embT1")]
                catT = chain.tile([P, H * DC, TQ], bf16)
                aoT = chain.tile([P, DC, TQ], bf16)
                hT = chain.tile([P, DC, TQ], bf16)

                # ---- stage 1: embedding + LN -> embTh (two halves of T) ----
                lng_sb = misc.tile([P, D], f32, tag="lng", name="lng_sb")
                lnb_sb = misc.tile([P, D], f32, tag="lnb", name="lnb_sb")
                for sb in range(T // P):
                    raw = tmp.tile([P, D], f32, tag="graw", bufs=4)
                    nc.gpsimd.indirect_dma_start(
                        out=raw, out_offset=None, in_=te_d[:],
                        in_offset=IndirectOffsetOnAxis(ap=idx_sb[:, sb:sb + 1], axis=0))
                    if sb == 0:
                        _bcast(nc, lng_sb, lng_d["e"], 0, D)
                        _bcast(nc, lnb_sb, lnb_d["e"], 0, D)
                    pe_blk = bc.tile([P, D], f32, tag="bc768")
                    nc.sync.dma_start(out=pe_blk, in_=peseg_r[:, sb])
                    nc.vector.tensor_add(raw, raw, pe_blk)
                    layernorm(tmp, raw, lng_sb, lnb_sb)
                    dstT, do = embTh[sb // 2], (sb % 2) * P
                    transpose_to(lambda dc, dstT=dstT, do=do:
                                 dstT[:, dc, do:do + P], raw)

                # deferred const loads (off the critical startup path)
                _bcast(nc, keep_sb, keep_d, 0, T)
                _bcast(nc, fill_sb, fill_d, 0, T)
                for k in ("q", "k"):
                    nc.sync.dma_start(out=bqk_sb[k], in_=bqk_r[k])
                nc.sync.dma_start(out=wcls_sb, in_=wc_r)
                _bcast(nc, bcls_sb, bc_d, 0, 2)

                # prefetch the first Wproj tile into the whole-kernel const pool
                # (never overlaps stage tiles, so no release anti-dependency)
                wtp0 = const.tile([P, DC, 512], bf16, name="wtp0")
                nc.sync.dma_start(out=wtp0, in_=wp_r[:, :, 0:512])
                pre_wt = [wtp0]

                # ---- stages 2+3: QKV + attention per head ----
                with tc.tile_pool(name="wqkv", bufs=3) as wqkv, \
                     tc.tile_pool(name="qkv", bufs=2) as qkv:
                    for h in range(H):
                        wq_sb = wqkv.tile([P, DC, DH], bf16, tag="wq")
                        nc.sync.dma_start(out=wq_sb, in_=wq_r[h])
                        wk_sb = wqkv.tile([P, DC, DH], bf16, tag="wk")
                        nc.gpsimd.dma_start(out=wk_sb, in_=wk_r[h])
                        wv_sb = wqkv.tile([P, DC, DH], bf16, tag="wv", bufs=2)
                        nc.sync.dma_start(out=wv_sb, in_=wv_r[h])

                        # QT [e, tq] with bias and sqrt-scale folded in
                        QT = qkv.tile([P, DC, TQ], bf16, tag="QT")
                        for ec in range(DC):
                            pq = psmid.tile([P, TQ], f32, tag="mid")
                            for dc in range(DC):
                                nc.tensor.matmul(pq, lhsT=wq_sb[:, dc, ec * P:(ec + 1) * P],
                                                 rhs=embTh[0][:, dc, :],
                                                 start=dc == 0, stop=dc == DC - 1)
                            # (psum + bq)*s == psum*s + bq*s; host pre-scales bq
                            nc.scalar.activation(
                                QT[:, ec], pq, AF.Identity, scale=SCALE,
                                bias=bqk_sb["q"][:, h, ec:ec + 1])
                        # KT [e, t] in two T-halves so half 0 starts early
                        KT = qkv.tile([P, DC, T], bf16, tag="KT")
                        for ec in range(DC):
                            for th in range(2):
                                pk = psbig.tile([P, TQ], f32, tag="big")
                                for dc in range(DC):
                                    nc.tensor.matmul(pk, lhsT=wk_sb[:, dc, ec * P:(ec + 1) * P],
                                                     rhs=embTh[th][:, dc, :],
                                                     start=dc == 0, stop=dc == DC - 1)
                                nc.scalar.activation(
                                    KT[:, ec, th * TQ:(th + 1) * TQ], pk,
                                    AF.Identity, bias=bqk_sb["k"][:, h, ec:ec + 1])
                        # V [s, e]
                        Vt = qkv.tile([P, T // P, DH], bf16, tag="V")
                        bv_bc = bc.tile([P, D], f32, tag="bc768")
                        _bcast(nc, bv_bc, bv_d, h * DH, DH)
                        for sb in range(T // P):
                            for eh in range(2):
                                pv = psmid.tile([P, 384], f32, tag="mid")
                                for dc in range(DC):
                                    nc.tensor.matmul(
                                        pv, lhsT=embTh[sb // 2][:, dc, (sb % 2) * P:(sb % 2) * P + P],
                                        rhs=wv_sb[:, dc, eh * 384:(eh + 1) * 384],
                                        start=dc == 0, stop=dc == DC - 1)
                                nc.vector.tensor_add(
                                    Vt[:, sb, eh * 384:(eh + 1) * 384], pv,
                                    bv_bc[:, eh * 384:(eh + 1) * 384])

                        atT = tmp.tile([P, T // P, TQ], bf16, tag="atT")
                        for tb in range(2):
                            toff = tb * P
                            ps_s = psbig.tile([P, T], f32, tag="big")
                            for ec in range(DC):
                                nc.tensor.matmul(ps_s, lhsT=QT[:, ec, toff:toff + P],
                                                 rhs=KT[:, ec, :],
                                                 start=ec == 0, stop=ec == DC - 1)
                            sc = tmp.tile([P, T], f32, tag="sc")
                            nc.vector.tensor_mul(sc, ps_s, keep_sb)
                            nc.vector.tensor_add(sc, sc, fill_sb)
                            nmax = tmp.tile([P, 1], f32, tag="nmax")
                            nc.vector.tensor_reduce(nmax, sc, axis=AX.X, op=OP.max,
                                                    negate=True)
                            ssum = tmp.tile([P, 1], f32, tag="ssum")
                            nc.scalar.activation(sc, sc, AF.Exp, bias=nmax,
                                                 accum_out=ssum)
                            rs = tmp.tile([P, 1], f32, tag="rs")
                            nc.vector.reciprocal(rs, ssum)
                            nc.vector.tensor_scalar_mul(sc, sc, scalar1=rs)
                            for s4 in range(T // P):
                                pt = pstr.tile([P, P], f32, tag="tr")
                                nc.tensor.transpose(pt, sc[:, s4 * P:(s4 + 1) * P], ident)
                                nc.vector.tensor_copy(atT[:, s4, toff:toff + P], pt)
                        for eb in range(DC):
                            pc = psmid.tile([P, TQ], f32, tag="mid")
                            for s4 in range(T // P):
                                nc.tensor.matmul(pc, lhsT=Vt[:, s4, eb * P:(eb + 1) * P],
                                                 rhs=atT[:, s4, :],
                                                 start=s4 == 0, stop=s4 == T // P - 1)
                            nc.vector.tensor_copy(catT[:, h * DC + eb, :], pc)

                # ---- stage 4: Wo + LN -> aoT;  stage 5: FFN;  stage 6: enc ----
                with tc.tile_pool(name="wbig", bufs=1) as wbig:
                    wo_sb = wbig.tile([P, H * DC, D], bf16)
                    nc.sync.dma_start(out=wo_sb[:, :H * DC // 2], in_=wo_r[:, :H * DC // 2])
                    nc.gpsimd.dma_start(out=wo_sb[:, H * DC // 2:], in_=wo_r[:, H * DC // 2:])
                    bo_bc = bc.tile([P, D], f32, tag="bc768")
                    _bcast(nc, bo_bc, bo_d, 0, D)
                    _bcast(nc, lng_sb, lng_d["a"], 0, D)
                    _bcast(nc, lnb_sb, lnb_d["a"], 0, D)
                    for tb in range(2):
                        toff = tb * P
                        ao = tmp.tile([P, D], f32, tag="raw")
                        for dh in range(2):
                            po = psmid.tile([P, 384], f32, tag="mid")
                            for hc in range(H * DC):
                                nc.tensor.matmul(po, lhsT=catT[:, hc, toff:toff + P],
                                                 rhs=wo_sb[:, hc, dh * 384:(dh + 1) * 384],
                                                 start=hc == 0, stop=hc == H * DC - 1)
                            nc.vector.tensor_add(ao[:, dh * 384:(dh + 1) * 384], po,
                                                 bo_bc[:, dh * 384:(dh + 1) * 384])
                        layernorm(tmp, ao, lng_sb, lnb_sb)
                        transpose_to(lambda dc, toff=toff: aoT[:, dc, toff:toff + P], ao)

                    w1_sb = wbig.tile([P, DC, DH], bf16, tag="w1")
                    nc.sync.dma_start(out=w1_sb, in_=w1_r)
                    w2_sb = wbig.tile([P, DC, D], bf16, tag="w2")
                    nc.gpsimd.dma_start(out=w2_sb, in_=w2_r)
                    b1_bc = bc.tile([P, D], f32, tag="bc768")
                    _bcast(nc, b1_bc, b1_d, 0, DH)
                    for tb in range(2):
                        toff = tb * P
                        hf = tmp.tile([P, DH], f32, tag="raw")
                        for eh in range(2):
                            ph = psmid.tile([P, 384], f32, tag="mid")
                            for dc in range(DC):
                                nc.tensor.matmul(ph, lhsT=aoT[:, dc, toff:toff + P],
                                                 rhs=w1_sb[:, dc, eh * 384:(eh + 1) * 384],
                                                 start=dc == 0, stop=dc == DC - 1)
                            nc.vector.tensor_add(hf[:, eh * 384:(eh + 1) * 384], ph,
                                                 b1_bc[:, eh * 384:(eh + 1) * 384])
                        hg = tmp.tile([P, DH], f32, tag="sc")
                        nc.scalar.activation(hg, hf, AF.Gelu)
                        transpose_to(lambda dc, toff=toff: hT[:, dc, toff:toff + P], hg)

                    b2_bc = bc.tile([P, D], f32, tag="bc768")
                    _bcast(nc, b2_bc, b2_d, 0, D)
                    _bcast(nc, lng_sb, lng_d["n"], 0, D)
                    _bcast(nc, lnb_sb, lnb_d["n"], 0, D)
                    for tb in range(2):
                        toff = tb * P
                        en = tmp.tile([P, D], f32, tag="raw")
                        for dh in range(2):
                            pe2 = psmid.tile([P, 384], f32, tag="mid")
                            for dc in range(DC):
                                nc.tensor.matmul(pe2, lhsT=hT[:, dc, toff:toff + P],
                                                 rhs=w2_sb[:, dc, dh * 384:(dh + 1) * 384],
                                                 start=dc == 0, stop=dc == DC - 1)
                            nc.vector.tensor_add(en[:, dh * 384:(dh + 1) * 384], pe2,
                                                 b2_bc[:, dh * 384:(dh + 1) * 384])
                        layernorm(tmp, en, lng_sb, lnb_sb)
                        transpose_to(lambda dc, tb=tb: encTh2[tb][:, dc, :], en)

                # ---- cls head (token 0 = row 0 of block 0 on q=0 cores) ----
                pc2 = psmid.tile([P, 2], f32, tag="mid")
                for dc in range(DC):
                    nc.tensor.matmul(pc2, lhsT=encTh2[0][:, dc, :], rhs=wcls_sb[:, dc, :],
                                     start=dc == 0, stop=dc == DC - 1)
                cls_sb = misc.tile([P, 2], f32)
                nc.vector.tensor_add(cls_sb, pc2, bcls_sb)
                nc.scalar.dma_start(out=cls_d[:], in_=cls_sb)

            # ---- stage 7: Wproj + log-softmax ----
            with tc.tile_pool(name="res", bufs=1) as resp, \
                 tc.tile_pool(name="bbf", bufs=2) as bbf, \
                 tc.tile_pool(name="scr", bufs=2) as scrp, \
                 tc.tile_pool(name="ostg", bufs=4) as ostg:
                res = [resp.tile([P, V], bf16, tag=f"res{tb}", name=f"res{tb}")
                       for tb in range(2)]
                bb = None
                BCH = 4096  # bias broadcast chunk (8 v-tiles)
                for nv in range(NV):
                    w = 512 if nv < NV - 1 else 256
                    off = nv * 512
                    if nv < 1:
                        wt = pre_wt[nv]
                    else:
                        wt = wstream.tile([P, DC, 512], bf16, tag="wt")
                        nc.sync.dma_start(out=wt[:, :, :w], in_=wp_r[:, :, off:off + w])
                    if off % BCH == 0:
                        bw = min(BCH, V - off)
                        bb = bbf.tile([P, BCH], bf16, tag="bb", name="bb")
                        nc.gpsimd.dma_start(out=bb[:, :bw],
                                            in_=bass.AP(bp_d, off, [[0, P], [1, bw]]))
                    boff = off % BCH
                    # second half: tb1 first, so tb0's lse finale starts early
                    for tb in ((0, 1) if nv < 32 else (1, 0)):
                        pp = psbig.tile([P, 512], f32, tag="big")
                        for dc in range(DC):
                            nc.tensor.matmul(pp[:, :w],
                                             lhsT=encTh2[tb][:, dc, :],
                                             rhs=wt[:, dc, :w],
                                             start=dc == 0, stop=dc == DC - 1)
                        nc.vector.tensor_add(res[tb][:, off:off + w], pp[:, :w],
                                             bb[:, boff:boff + w])
                        scr = scrp.tile([P, 512], f32, tag="scr")
                        nc.scalar.activation(scr[:, :w], res[tb][:, off:off + w],
                                             AF.Exp, accum_out=sums[:, tb, nv:nv + 1])
                for tb in range(2):
                    tot = misc.tile([P, 1], f32, tag=f"tot{tb}")
                    nc.vector.reduce_sum(tot, sums[:, tb, :NV], axis=AX.X)
                    nlse = misc.tile([P, 1], f32, tag=f"nlse{tb}")
                    nc.scalar.activation(nlse, tot, AF.Ln)
                    nc.vector.tensor_scalar_mul(nlse, nlse, -1.0)
                    CH = 2000
                    for ch in range(V // CH):
                        o = ostg.tile([P, CH], mybir.dt.float16, tag="o")
                        src = res[tb][:, ch * CH:(ch + 1) * CH]
                        r = ch % 3
                        if r == 0:
                            nc.vector.tensor_scalar_add(o, src, scalar1=nlse)
                        elif r == 1:
                            nc.scalar.activation(o, src, AF.Identity, bias=nlse)
                        else:
                            nc.gpsimd.tensor_scalar_add(o, src, scalar1=nlse)
                        nc.sync.dma_start(
                            out=logits_d[tb * P:(tb + 1) * P, ch * CH:(ch + 1) * CH],
                            in_=o)
            ws_ctx.__exit__(None, None, None)

    nc.finalize()
    return nc


_NC = None


def _get_nc():
    global _NC
    if _NC is None:
        _NC = build_program()
    return _NC


def kernel(**inputs):
    x = np.asarray(inputs["x"])
    amask = np.asarray(inputs["attention_mask"])
    g = {k: np.ascontiguousarray(np.asarray(v)) for k, v in inputs.items()}

    bfc = lambda a: np.ascontiguousarray(np.asarray(a)).astype(ml_dtypes.bfloat16)
    f32c = lambda a: np.ascontiguousarray(np.asarray(a, dtype=np.float32))

    # positional encoding + segment rows (identical across batches)
    d = (2.0 * np.arange(D, dtype=np.float32) / np.float32(D)).astype(np.float32)
    denom = np.power(np.float32(10000.0), d, dtype=np.float32)
    p = np.arange(T, dtype=np.float32)[:, None] / denom[None, :]
    pe = np.where(np.arange(D) % 2 == 0, np.sin(p), np.cos(p)).astype(np.float32)
    seg = f32c(g["segment_embed"])
    peseg = pe.copy()
    peseg[: T // 2 + 1] += seg[0]
    peseg[T // 2 + 1:] += seg[1]

    shared = {
        "te": f32c(g["token_embed"]),
        "peseg": peseg,
        "wq": bfc(g["Wq"]), "wk": bfc(g["Wk"]), "wv": bfc(g["Wv"]),
        "bq": f32c(g["bq"]) * np.float32(SCALE), "bk": f32c(g["bk"]),
        "bv": f32c(g["bv"]),
        "wo": bfc(g["Wo"]), "bo": f32c(g["bo"]),
        "w1": bfc(g["W1"]), "b1": f32c(g["b1"]),
        "w2": bfc(g["W2"]), "b2": f32c(g["b2"]),
        "lneg": f32c(g["ln_embed_g"]), "lneb": f32c(g["ln_embed_b"]),
        "lnag": f32c(g["ln_attn_g"]), "lnab": f32c(g["ln_attn_b"]),
        "lnng": f32c(g["ln_enc_g"]), "lnnb": f32c(g["ln_enc_b"]),
        "wp": bfc(g["Wproj"]), "bp": bfc(g["bproj"]),
        "wc": bfc(g["Wcls"]), "bc": f32c(g["bcls"]),
        "eye": np.eye(P, dtype=np.float32),
    }

    in_maps = []
    for c in range(8):
        b, q = c // 2, c % 2
        roll = -q * TQ  # q=1 cores see tokens rolled so queries sit at 0..255
        idx_c = np.roll(x[b].astype(np.int32), roll)
        m = np.roll(amask[b, 0].astype(np.float32), roll)
        peseg_c = np.roll(peseg, roll, axis=0) if q else peseg
        im = dict(shared)
        im["idx"] = np.ascontiguousarray(idx_c)
        im["peseg"] = np.ascontiguousarray(peseg_c)
        im["keep"] = np.ascontiguousarray(1.0 - m)
        im["fill"] = np.ascontiguousarray(m * np.float32(1e-9))
        in_maps.append(im)

    nc = _get_nc()
    import os
    trace = bool(os.environ.get("BERT_TRACE"))
    kw = {}
    if trace:
        kw = dict(trace=True, tmpdir=os.environ.get("BERT_TRACE_DIR") or None)
    res = run_bass_kernel_spmd(nc, in_maps, list(range(8)), **kw)
    if trace:
        print("HW exec_time_ns:", res.exec_time_ns)

    logits = np.empty((B, T, V), dtype=np.float32)
    cls = np.empty((B, 2), dtype=np.float32)
    for c in range(8):
        b, q = c // 2, c % 2
        logits[b, q * TQ:(q + 1) * TQ] = res.results[c]["logits"].astype(np.float32)
        if q == 0:
            cls[b] = res.results[c]["cls"][0]
    return logits, cls


# revision 82
# speedup vs baseline: 1.0026x; 1.0026x over previous
"""BERT single-layer kernel for 8 Trainium2 NeuronCores.

Sharding: core c handles batch b=c//2, query-token half q=c%2 (256 of 512
tokens). Each core computes K/V for its batch's full 512 tokens (duplicated
within the pair, avoiding collectives). The dominant enc@Wproj [256,768]x
[768,32000] streams bf16 weights once; both 128-token blocks' logits live in
SBUF bf16, log-softmax is computed online via per-tile exp row-sums.
"""
import sys

sys.path.insert(0, "/opt/trn_rl_repo")

import numpy as np
import ml_dtypes

import concourse.bass as bass
import concourse.bacc as bacc
import concourse.mybir as mybir
import concourse.tile as tile
from concourse.bass import IndirectOffsetOnAxis
from concourse.bass_utils import run_bass_kernel_spmd

P = 128
B, T, V, D, DH, H = 4, 512, 32000, 768, 768, 4
DC = D // P          # 6 chunks of the feature dim
TQ = 256             # query tokens per core
NV = 63              # vocab tiles: 62*512 + 256
SCALE = float(T) ** -0.5
EPS = 1e-5

f32 = mybir.dt.float32
bf16 = mybir.dt.bfloat16
i32 = mybir.dt.int32

AF = mybir.ActivationFunctionType
AX = mybir.AxisListType
OP = mybir.AluOpType


def _bcast(nc, out_tile, dram, offset, width):
    """Broadcast `width` contiguous elements at `offset` of 1-D dram tensor
    across all 128 partitions of out_tile."""
    nc.gpsimd.dma_start(out=out_tile, in_=bass.AP(dram, offset, [[0, P], [1, width]]))


def build_program():
    nc = bacc.Bacc(None)

    # ---- dram I/O ----
    idx_d = nc.dram_tensor("idx", [T], i32, kind="ExternalInput")
    te_d = nc.dram_tensor("te", [V, D], f32, kind="ExternalInput")
    peseg_d = nc.dram_tensor("peseg", [T, D], f32, kind="ExternalInput")
    keep_d = nc.dram_tensor("keep", [T], f32, kind="ExternalInput")
    fill_d = nc.dram_tensor("fill", [T], f32, kind="ExternalInput")
    wq_d = nc.dram_tensor("wq", [H, D, DH], bf16, kind="ExternalInput")
    wk_d = nc.dram_tensor("wk", [H, D, DH], bf16, kind="ExternalInput")
    wv_d = nc.dram_tensor("wv", [H, D, DH], bf16, kind="ExternalInput")
    bq_d = nc.dram_tensor("bq", [H, DH], f32, kind="ExternalInput")
    bk_d = nc.dram_tensor("bk", [H, DH], f32, kind="ExternalInput")
    bv_d = nc.dram_tensor("bv", [H, DH], f32, kind="ExternalInput")
    wo_d = nc.dram_tensor("wo", [H * DH, D], bf16, kind="ExternalInput")
    bo_d = nc.dram_tensor("bo", [D], f32, kind="ExternalInput")
    w1_d = nc.dram_tensor("w1", [D, DH], bf16, kind="ExternalInput")
    b1_d = nc.dram_tensor("b1", [DH], f32, kind="ExternalInput")
    w2_d = nc.dram_tensor("w2", [DH, D], bf16, kind="ExternalInput")
    b2_d = nc.dram_tensor("b2", [D], f32, kind="ExternalInput")
    lng_d = {k: nc.dram_tensor(f"ln{k}g", [D], f32, kind="ExternalInput")
             for k in ("e", "a", "n")}
    lnb_d = {k: nc.dram_tensor(f"ln{k}b", [D], f32, kind="ExternalInput")
             for k in ("e", "a", "n")}
    wp_d = nc.dram_tensor("wp", [D, V], bf16, kind="ExternalInput")
    bp_d = nc.dram_tensor("bp", [V], bf16, kind="ExternalInput")
    wc_d = nc.dram_tensor("wc", [D, 2], bf16, kind="ExternalInput")
    bc_d = nc.dram_tensor("bc", [2], f32, kind="ExternalInput")
    eye_d = nc.dram_tensor("eye", [P, P], f32, kind="ExternalInput")

    logits_d = nc.dram_tensor("logits", [TQ, V], mybir.dt.float16,
                              kind="ExternalOutput")
    cls_d = nc.dram_tensor("cls", [P, 2], f32, kind="ExternalOutput")

    # SPMD: one program for all 8 cores. Query tokens always sit at positions
    # 0..255 — the host rolls token order by -256 for q=1 cores (attention is
    # permutation-invariant over the key/value axis since mask/pe roll along).

    wq_r = [wq_d[h].rearrange("(dc p) e -> p dc e", p=P) for h in range(H)]
    wk_r = [wk_d[h].rearrange("(dc p) e -> p dc e", p=P) for h in range(H)]
    wv_r = [wv_d[h].rearrange("(dc p) e -> p dc e", p=P) for h in range(H)]
    wo_r = wo_d.rearrange("(hc p) d -> p hc d", p=P)
    w1_r = w1_d.rearrange("(dc p) e -> p dc e", p=P)
    w2_r = w2_d.rearrange("(dc p) e -> p dc e", p=P)
    wp_r = wp_d.rearrange("(dc p) v -> p dc v", p=P)
    wc_r = wc_d.rearrange("(dc p) c -> p dc c", p=P)
    idx_r = idx_d.rearrange("(sb p) -> p sb", p=P)
    peseg_r = peseg_d.rearrange("(sb p) d -> p sb d", p=P)
    bqk_r = {"q": bq_d.rearrange("h (ec p) -> p h ec", p=P),
             "k": bk_d.rearrange("h (ec p) -> p h ec", p=P)}

    with tile.TileContext(nc) as tc:
        with tc.tile_pool(name="const", bufs=1) as const, \
             tc.tile_pool(name="misc", bufs=1) as misc, \
             tc.tile_pool(name="enc", bufs=1) as encp, \
             tc.tile_pool(name="psbig", bufs=4, space="PSUM") as psbig, \
             tc.tile_pool(name="psmid", bufs=2, space="PSUM") as psmid, \
             tc.tile_pool(name="pstr", bufs=2, space="PSUM") as pstr:

            idx_sb = const.tile([P, T // P], i32)
            nc.sync.dma_start(out=idx_sb, in_=idx_r)
            ident = const.tile([P, P], f32)
            nc.sync.dma_start(out=ident, in_=eye_d[:])
            eps_t = const.tile([P, 1], f32)
            nc.vector.memset(eps_t, EPS)
            # dummy op: pull the sqrt ACT table load off the first LN's path
            warm = const.tile([P, 1], f32)
            nc.scalar.activation(warm, eps_t, AF.Sqrt)
            keep_sb = const.tile([P, T], f32)
            fill_sb = const.tile([P, T], f32)
            bqk_sb = {}
            for k in ("q", "k"):
                bqk_sb[k] = const.tile([P, H, DC], f32, name=f"b{k}sb", tag=f"b{k}sb")
            wcls_sb = const.tile([P, DC, 2], bf16)
            bcls_sb = const.tile([P, 2], f32)

            encTh2 = [encp.tile([P, DC, P], bf16, tag="encT0", name="encT0"),
                      encp.tile([P, DC, P], bf16, tag="encT1", name="encT1")]
            sums = misc.tile([P, 2, 64], f32)


            def layernorm(tp, x, g_bc, b_bc, lnexp=False):
                """LN over free axis of x [P, D] f32, in place (then *g+b).

                lnexp=True computes rsqrt as exp(-0.5*ln(v+eps)) — keeps ACT in
                the exp/ln table set (no sqrt-set swap) and frees DVE of the
                reciprocal; used for the LNs adjacent to exp/gelu phases."""
                st = tp.tile([P, 3, 6], f32, tag="ln_st")
                for i in range(3):
                    nc.vector.bn_stats(out=st[:, i], in_=x[:, i * 256:(i + 1) * 256])
                mv = tp.tile([P, 2], f32, tag="ln_mv")
                nc.vector.bn_aggr(out=mv, in_=st)
                rstd = tp.tile([P, 1], f32, tag="ln_rstd")
                if lnexp:
                    nc.scalar.activation(rstd, mv[:, 1:2], AF.Ln, bias=eps_t)
                    nc.scalar.activation(rstd, rstd, AF.Exp, scale=-0.5)
                else:
                    nc.scalar.activation(rstd, mv[:, 1:2], AF.Sqrt, bias=eps_t)
                    nc.vector.reciprocal(rstd, rstd)
                nc.vector.tensor_scalar(x, x, scalar1=mv[:, 0:1], scalar2=rstd,
                                        op0=OP.subtract, op1=OP.mult)
                nc.vector.tensor_mul(x, x, g_bc)
                nc.vector.tensor_add(x, x, b_bc)

            def transpose_to(dst_slices, x):
                """x [P(tok), D] f32 -> dst[:, dc, tok-slice] bf16 transposed.

                PSUM->SBUF copies go to ACT (idle in the LN stages) so they
                run parallel to the DVE layernorm chains."""
                for dc in range(DC):
                    pt = pstr.tile([P, P], f32, tag="tr")
                    nc.tensor.transpose(pt, x[:, dc * P:(dc + 1) * P], ident)
                    nc.scalar.copy(dst_slices(dc), pt)

            ws_ctx = tc.tile_pool(name="wstream", bufs=3)
            wstream = ws_ctx.__enter__()
            with tc.tile_pool(name="chain", bufs=1) as chain, \
                 tc.tile_pool(name="bc", bufs=3) as bc, \
                 tc.tile_pool(name="tmp", bufs=2) as tmp:

                embTh = [chain.tile([P, DC, TQ], bf16, tag="embT0", name="embT0"),
                         chain.tile([P, DC, TQ], bf16, tag="embT1", name="embT1")]
                catT = chain.tile([P, H * DC, TQ], bf16)
                aoT = chain.tile([P, DC, TQ], bf16)
                hT = chain.tile([P, DC, TQ], bf16)

                # ---- stage 1: embedding + LN -> embTh (two halves of T) ----
                lng_sb = misc.tile([P, D], f32, tag="lng", name="lng_sb")
                lnb_sb = misc.tile([P, D], f32, tag="lnb", name="lnb_sb")
                for sb in range(T // P):
                    raw = tmp.tile([P, D], f32, tag="graw", bufs=4)
                    nc.gpsimd.indirect_dma_start(
                        out=raw, out_offset=None, in_=te_d[:],
                        in_offset=IndirectOffsetOnAxis(ap=idx_sb[:, sb:sb + 1], axis=0))
                    if sb == 0:
                        _bcast(nc, lng_sb, lng_d["e"], 0, D)
                        _bcast(nc, lnb_sb, lnb_d["e"], 0, D)
                    pe_blk = bc.tile([P, D], f32, tag="bc768")
                    nc.sync.dma_start(out=pe_blk, in_=peseg_r[:, sb])
                    nc.gpsimd.tensor_add(raw, raw, pe_blk)
                    layernorm(tmp, raw, lng_sb, lnb_sb)
                    dstT, do = embTh[sb // 2], (sb % 2) * P
                    transpose_to(lambda dc, dstT=dstT, do=do:
                                 dstT[:, dc, do:do + P], raw)

                # deferred const loads (off the critical startup path)
                _bcast(nc, keep_sb, keep_d, 0, T)
                _bcast(nc, fill_sb, fill_d, 0, T)
                for k in ("q", "k"):
                    nc.sync.dma_start(out=bqk_sb[k], in_=bqk_r[k])
                nc.sync.dma_start(out=wcls_sb, in_=wc_r)
                _bcast(nc, bcls_sb, bc_d, 0, 2)

                # prefetch the first Wproj tile into the whole-kernel const pool
                # (never overlaps stage tiles, so no release anti-dependency)
                wtp0 = const.tile([P, DC, 512], bf16, name="wtp0")
                nc.sync.dma_start(out=wtp0, in_=wp_r[:, :, 0:512])
                pre_wt = [wtp0]

                # ---- stages 2+3: QKV + attention per head ----
                with tc.tile_pool(name="wqkv", bufs=3) as wqkv, \
                     tc.tile_pool(name="qkv", bufs=2) as qkv:
                    for h in range(H):
                        wq_sb = wqkv.tile([P, DC, DH], bf16, tag="wq")
                        nc.sync.dma_start(out=wq_sb, in_=wq_r[h])
                        wk_sb = wqkv.tile([P, DC, DH], bf16, tag="wk")
                        nc.gpsimd.dma_start(out=wk_sb, in_=wk_r[h])
                        wv_sb = wqkv.tile([P, DC, DH], bf16, tag="wv", bufs=2)
                        nc.sync.dma_start(out=wv_sb, in_=wv_r[h])

                        # QT [e, tq] with bias and sqrt-scale folded in
                        QT = qkv.tile([P, DC, TQ], bf16, tag="QT")
                        for ec in range(DC):
                            pq = psmid.tile([P, TQ], f32, tag="mid")
                            for dc in range(DC):
                                nc.tensor.matmul(pq, lhsT=wq_sb[:, dc, ec * P:(ec + 1) * P],
                                                 rhs=embTh[0][:, dc, :],
                                                 start=dc == 0, stop=dc == DC - 1)
                            # (psum + bq)*s == psum*s + bq*s; host pre-scales bq
                            nc.scalar.activation(
                                QT[:, ec], pq, AF.Identity, scale=SCALE,
                                bias=bqk_sb["q"][:, h, ec:ec + 1])
                        # KT [e, t] in two T-halves so half 0 starts early
                        KT = qkv.tile([P, DC, T], bf16, tag="KT")
                        for ec in range(DC):
                            for th in range(2):
                                pk = psbig.tile([P, TQ], f32, tag="big")
                                for dc in range(DC):
                                    nc.tensor.matmul(pk, lhsT=wk_sb[:, dc, ec * P:(ec + 1) * P],
                                                     rhs=embTh[th][:, dc, :],
                                                     start=dc == 0, stop=dc == DC - 1)
                                nc.scalar.activation(
                                    KT[:, ec, th * TQ:(th + 1) * TQ], pk,
                                    AF.Identity, bias=bqk_sb["k"][:, h, ec:ec + 1])
                        # V [s, e]
                        Vt = qkv.tile([P, T // P, DH], bf16, tag="V")
                        bv_bc = bc.tile([P, D], f32, tag="bc768")
                        _bcast(nc, bv_bc, bv_d, h * DH, DH)
                        for sb in range(T // P):
                            for eh in range(2):
                                pv = psmid.tile([P, 384], f32, tag="mid")
                                for dc in range(DC):
                                    nc.tensor.matmul(
                                        pv, lhsT=embTh[sb // 2][:, dc, (sb % 2) * P:(sb % 2) * P + P],
                                        rhs=wv_sb[:, dc, eh * 384:(eh + 1) * 384],
                                        start=dc == 0, stop=dc == DC - 1)
                                nc.vector.tensor_add(
                                    Vt[:, sb, eh * 384:(eh + 1) * 384], pv,
                                    bv_bc[:, eh * 384:(eh + 1) * 384])

                        atT = tmp.tile([P, T // P, TQ], bf16, tag="atT")
                        for tb in range(2):
                            toff = tb * P
                            ps_s = psbig.tile([P, T], f32, tag="big")
                            for ec in range(DC):
                                nc.tensor.matmul(ps_s, lhsT=QT[:, ec, toff:toff + P],
                                                 rhs=KT[:, ec, :],
                                                 start=ec == 0, stop=ec == DC - 1)
                            sc = tmp.tile([P, T], f32, tag="sc")
                            nc.vector.tensor_mul(sc, ps_s, keep_sb)
                            nc.vector.tensor_add(sc, sc, fill_sb)
                            nmax = tmp.tile([P, 1], f32, tag="nmax")
                            nc.vector.tensor_reduce(nmax, sc, axis=AX.X, op=OP.max,
                                                    negate=True)
                            ssum = tmp.tile([P, 1], f32, tag="ssum")
                            nc.scalar.activation(sc, sc, AF.Exp, bias=nmax,
                                                 accum_out=ssum)
                            rs = tmp.tile([P, 1], f32, tag="rs")
                            nc.vector.reciprocal(rs, ssum)
                            nc.vector.tensor_scalar_mul(sc, sc, scalar1=rs)
                            for s4 in range(T // P):
                                pt = pstr.tile([P, P], f32, tag="tr")
                                nc.tensor.transpose(pt, sc[:, s4 * P:(s4 + 1) * P], ident)
                                nc.vector.tensor_copy(atT[:, s4, toff:toff + P], pt)
                        for eb in range(DC):
                            pc = psmid.tile([P, TQ], f32, tag="mid")
                            for s4 in range(T // P):
                                nc.tensor.matmul(pc, lhsT=Vt[:, s4, eb * P:(eb + 1) * P],
                                                 rhs=atT[:, s4, :],
                                                 start=s4 == 0, stop=s4 == T // P - 1)
                            nc.vector.tensor_copy(catT[:, h * DC + eb, :], pc)

                # ---- stage 4: Wo + LN -> aoT;  stage 5: FFN;  stage 6: enc ----
                with tc.tile_pool(name="wbig", bufs=1) as wbig:
                    wo_sb = wbig.tile([P, H * DC, D], bf16)
                    nc.sync.dma_start(out=wo_sb[:, :H * DC // 2], in_=wo_r[:, :H * DC // 2])
                    nc.gpsimd.dma_start(out=wo_sb[:, H * DC // 2:], in_=wo_r[:, H * DC // 2:])
                    bo_bc = bc.tile([P, D], f32, tag="bc768")
                    _bcast(nc, bo_bc, bo_d, 0, D)
                    _bcast(nc, lng_sb, lng_d["a"], 0, D)
                    _bcast(nc, lnb_sb, lnb_d["a"], 0, D)
                    for tb in range(2):
                        toff = tb * P
                        ao = tmp.tile([P, D], f32, tag="raw")
                        for dh in range(2):
                            po = psmid.tile([P, 384], f32, tag="mid")
                            for hc in range(H * DC):
                                nc.tensor.matmul(po, lhsT=catT[:, hc, toff:toff + P],
                                                 rhs=wo_sb[:, hc, dh * 384:(dh + 1) * 384],
                                                 start=hc == 0, stop=hc == H * DC - 1)
                            nc.vector.tensor_add(ao[:, dh * 384:(dh + 1) * 384], po,
                                                 bo_bc[:, dh * 384:(dh + 1) * 384])
                        layernorm(tmp, ao, lng_sb, lnb_sb)
                        transpose_to(lambda dc, toff=toff: aoT[:, dc, toff:toff + P], ao)

                    w1_sb = wbig.tile([P, DC, DH], bf16, tag="w1")
                    nc.sync.dma_start(out=w1_sb, in_=w1_r)
                    w2_sb = wbig.tile([P, DC, D], bf16, tag="w2")
                    nc.gpsimd.dma_start(out=w2_sb, in_=w2_r)
                    b1_bc = bc.tile([P, D], f32, tag="bc768")
                    _bcast(nc, b1_bc, b1_d, 0, DH)
                    for tb in range(2):
                        toff = tb * P
                        hf = tmp.tile([P, DH], f32, tag="raw")
                        for eh in range(2):
                            ph = psmid.tile([P, 384], f32, tag="mid")
                            for dc in range(DC):
                                nc.tensor.matmul(ph, lhsT=aoT[:, dc, toff:toff + P],
                                                 rhs=w1_sb[:, dc, eh * 384:(eh + 1) * 384],
                                                 start=dc == 0, stop=dc == DC - 1)
                            nc.vector.tensor_add(hf[:, eh * 384:(eh + 1) * 384], ph,
                                                 b1_bc[:, eh * 384:(eh + 1) * 384])
                        hg = tmp.tile([P, DH], f32, tag="sc")
                        nc.scalar.activation(hg, hf, AF.Gelu)
                        transpose_to(lambda dc, toff=toff: hT[:, dc, toff:toff + P], hg)

                    b2_bc = bc.tile([P, D], f32, tag="bc768")
                    _bcast(nc, b2_bc, b2_d, 0, D)
                    _bcast(nc, lng_sb, lng_d["n"], 0, D)
                    _bcast(nc, lnb_sb, lnb_d["n"], 0, D)
                    for tb in range(2):
                        toff = tb * P
                        en = tmp.tile([P, D], f32, tag="raw")
                        for dh in range(2):
                            pe2 = psmid.tile([P, 384], f32, tag="mid")
                            for dc in range(DC):
                                nc.tensor.matmul(pe2, lhsT=hT[:, dc, toff:toff + P],
                                                 rhs=w2_sb[:, dc, dh * 384:(dh + 1) * 384],
                                                 start=dc == 0, stop=dc == DC - 1)
                            nc.vector.tensor_add(en[:, dh * 384:(dh + 1) * 384], pe2,
                                                 b2_bc[:, dh * 384:(dh + 1) * 384])
                        layernorm(tmp, en, lng_sb, lnb_sb)
                        transpose_to(lambda dc, tb=tb: encTh2[tb][:, dc, :], en)

                # ---- cls head (token 0 = row 0 of block 0 on q=0 cores) ----
                pc2 = psmid.tile([P, 2], f32, tag="mid")
                for dc in range(DC):
                    nc.tensor.matmul(pc2, lhsT=encTh2[0][:, dc, :], rhs=wcls_sb[:, dc, :],
                                     start=dc == 0, stop=dc == DC - 1)
                cls_sb = misc.tile([P, 2], f32)
                nc.vector.tensor_add(cls_sb, pc2, bcls_sb)
                nc.scalar.dma_start(out=cls_d[:], in_=cls_sb)

            # ---- stage 7: Wproj + log-softmax ----
            with tc.tile_pool(name="res", bufs=1) as resp, \
                 tc.tile_pool(name="bbf", bufs=2) as bbf, \
                 tc.tile_pool(name="scr", bufs=2) as scrp, \
                 tc.tile_pool(name="ostg", bufs=4) as ostg:
                res = [resp.tile([P, V], bf16, tag=f"res{tb}", name=f"res{tb}")
                       for tb in range(2)]
                bb = None
                BCH = 4096  # bias broadcast chunk (8 v-tiles)
                for nv in range(NV):
                    w = 512 if nv < NV - 1 else 256
                    off = nv * 512
                    if nv < 1:
                        wt = pre_wt[nv]
                    else:
                        wt = wstream.tile([P, DC, 512], bf16, tag="wt")
                        nc.sync.dma_start(out=wt[:, :, :w], in_=wp_r[:, :, off:off + w])
                    if off % BCH == 0:
                        bw = min(BCH, V - off)
                        bb = bbf.tile([P, BCH], bf16, tag="bb", name="bb")
                        nc.gpsimd.dma_start(out=bb[:, :bw],
                                            in_=bass.AP(bp_d, off, [[0, P], [1, bw]]))
                    boff = off % BCH
                    # second half: tb1 first, so tb0's lse finale starts early
                    for tb in ((0, 1) if nv < 32 else (1, 0)):
                        pp = psbig.tile([P, 512], f32, tag="big")
                        for dc in range(DC):
                            nc.tensor.matmul(pp[:, :w],
                                             lhsT=encTh2[tb][:, dc, :],
                                             rhs=wt[:, dc, :w],
                                             start=dc == 0, stop=dc == DC - 1)
                        nc.vector.tensor_add(res[tb][:, off:off + w], pp[:, :w],
                                             bb[:, boff:boff + w])
                        scr = scrp.tile([P, 512], f32, tag="scr")
                        nc.scalar.activation(scr[:, :w], res[tb][:, off:off + w],
                                             AF.Exp, accum_out=sums[:, tb, nv:nv + 1])
                for tb in range(2):
                    tot = misc.tile([P, 1], f32, tag=f"tot{tb}")
                    nc.vector.reduce_sum(tot, sums[:, tb, :NV], axis=AX.X)
                    nlse = misc.tile([P, 1], f32, tag=f"nlse{tb}")
                    nc.scalar.activation(nlse, tot, AF.Ln)
                    nc.vector.tensor_scalar_mul(nlse, nlse, -1.0)
                    CH = 2000
                    for ch in range(V // CH):
                        o = ostg.tile([P, CH], mybir.dt.float16, tag="o")
                        src = res[tb][:, ch * CH:(ch + 1) * CH]
                        r = ch % 3
                        if r == 0:
                            nc.vector.tensor_scalar_add(o, src, scalar1=nlse)
                        elif r == 1:
                            nc.scalar.activation(o, src, AF.Identity, bias=nlse)
                        else:
                            nc.gpsimd.tensor_scalar_add(o, src, scalar1=nlse)
                        nc.sync.dma_start(
                            out=logits_d[tb * P:(tb + 1) * P, ch * CH:(ch + 1) * CH],
                            in_=o)
            ws_ctx.__exit__(None, None, None)

    nc.finalize()
    return nc


_NC = None


def _get_nc():
    global _NC
    if _NC is None:
        _NC = build_program()
    return _NC


def kernel(**inputs):
    x = np.asarray(inputs["x"])
    amask = np.asarray(inputs["attention_mask"])
    g = {k: np.ascontiguousarray(np.asarray(v)) for k, v in inputs.items()}

    bfc = lambda a: np.ascontiguousarray(np.asarray(a)).astype(ml_dtypes.bfloat16)
    f32c = lambda a: np.ascontiguousarray(np.asarray(a, dtype=np.float32))

    # positional encoding + segment rows (identical across batches)
    d = (2.0 * np.arange(D, dtype=np.float32) / np.float32(D)).astype(np.float32)
    denom = np.power(np.float32(10000.0), d, dtype=np.float32)
    p = np.arange(T, dtype=np.float32)[:, None] / denom[None, :]
    pe = np.where(np.arange(D) % 2 == 0, np.sin(p), np.cos(p)).astype(np.float32)
    seg = f32c(g["segment_embed"])
    peseg = pe.copy()
    peseg[: T // 2 + 1] += seg[0]
    peseg[T // 2 + 1:] += seg[1]

    shared = {
        "te": f32c(g["token_embed"]),
        "peseg": peseg,
        "wq": bfc(g["Wq"]), "wk": bfc(g["Wk"]), "wv": bfc(g["Wv"]),
        "bq": f32c(g["bq"]) * np.float32(SCALE), "bk": f32c(g["bk"]),
        "bv": f32c(g["bv"]),
        "wo": bfc(g["Wo"]), "bo": f32c(g["bo"]),
        "w1": bfc(g["W1"]), "b1": f32c(g["b1"]),
        "w2": bfc(g["W2"]), "b2": f32c(g["b2"]),
        "lneg": f32c(g["ln_embed_g"]), "lneb": f32c(g["ln_embed_b"]),
        "lnag": f32c(g["ln_attn_g"]), "lnab": f32c(g["ln_attn_b"]),
        "lnng": f32c(g["ln_enc_g"]), "lnnb": f32c(g["ln_enc_b"]),
        "wp": bfc(g["Wproj"]), "bp": bfc(g["bproj"]),
        "wc": bfc(g["Wcls"]), "bc": f32c(g["bcls"]),
        "eye": np.eye(P, dtype=np.float32),
    }

    in_maps = []
    for c in range(8):
        b, q = c // 2, c % 2
        roll = -q * TQ  # q=1 cores see tokens rolled so queries sit at 0..255
        idx_c = np.roll(x[b].astype(np.int32), roll)
        m = np.roll(amask[b, 0].astype(np.float32), roll)
        peseg_c = np.roll(peseg, roll, axis=0) if q else peseg
        im = dict(shared)
        im["idx"] = np.ascontiguousarray(idx_c)
        im["peseg"] = np.ascontiguousarray(peseg_c)
        im["keep"] = np.ascontiguousarray(1.0 - m)
        im["fill"] = np.ascontiguousarray(m * np.float32(1e-9))
        in_maps.append(im)

    nc = _get_nc()
    import os
    trace = bool(os.environ.get("BERT_TRACE"))
    kw = {}
    if trace:
        kw = dict(trace=True, tmpdir=os.environ.get("BERT_TRACE_DIR") or None)
    res = run_bass_kernel_spmd(nc, in_maps, list(range(8)), **kw)
    if trace:
        print("HW exec_time_ns:", res.exec_time_ns)

    logits = np.empty((B, T, V), dtype=np.float32)
    cls = np.empty((B, 2), dtype=np.float32)
    for c in range(8):
        b, q = c // 2, c % 2
        logits[b, q * TQ:(q + 1) * TQ] = res.results[c]["logits"].astype(np.float32)
        if q == 0:
            cls[b] = res.results[c]["cls"][0]
    return logits, cls
